# revision 29
# baseline (speedup 1.0000x reference)
"""Trainium2 Bass kernel for GQA attention with RoPE (tensor-parallel over heads).

Reference computation (per problem spec):
  x:[1,2048,4096], wq:[4096,4096], wk/wv:[4096,1024], wo:[4096,4096], f32
  q/k/v proj -> RoPE(q,k) -> causal GQA softmax attention -> o_proj

Sharding: 8 cores, tensor-parallel over heads. Core c gets 4 query heads
(wq cols [c*512:(c+1)*512]) and 1 KV head (wk/wv cols [c*128:(c+1)*128]),
plus wo rows [c*512:(c+1)*512]. Each core computes a full [2048,4096]
partial o_proj output; the host sums the 8 partials (the all-reduce).
The host dispatch layer hands the device x pre-transposed ([D,S]).

Schedule (v2 of this kernel): the attention inner loop is co-limited by
ScalarE exp throughput ((N+352)/1.2 ns per ACT) -- attention PE work per
q-tile is smaller than its exp stream.  So o_proj for q-tile qi-1 is
interleaved unit-by-unit (one (si,mi) PSUM block = 4 matmuls) into the
attention pair loop of q-tile qi, keeping PE busy while exps stream.
The softmax denominator is NOT computed with per-pair ones-matmuls (that
costs a full extra pass of PE columns); instead exp'd probabilities are
accumulated on DVE (pacc2) and reduced across partitions once per
(head, q-tile) with a gpsimd partition_all_reduce (which also replaces
the partition_broadcast).  PV runs one pair behind QK so exp latency
hides under PE work; PSUM = st2(4 banks) + oacc(2) + opj(2) = 8.

Matmul operands are fp16 (see kernel_f32r.py note in the repo history:
16-bit weights get Fast-Weight-Load hidden under the matmul stream).
All accumulation stays fp32 in PSUM; softmax statistics are fp16 partial
sums reduced in fp32 by gpsimd. exp uses a constant bias (exp(s-10))
that cancels in the normalization.
"""
import numpy as np

import concourse.bass as bass
import concourse.bacc as bacc
import concourse.tile as tile
import concourse.mybir as mybir
from concourse import bass_utils
from concourse import bass_isa

F32 = mybir.dt.float32
F16 = mybir.dt.float16
AF = mybir.ActivationFunctionType

# model dims (hardcoded per problem spec nn_Attention_52020643889298)
S = 2048
D = 4096
H = 32
KV = 8
HD = 128
THETA = 10000.0
NCORES = 8
HQ = H // NCORES            # 4 query heads per core
NQ = HQ * HD                # 512 wq cols per core
NKV = (KV // NCORES) * HD   # 128 wk/wv cols per core

# tiling
SSTRIP = 512                # phase-1 s-strip
NSTRIPS = S // SSTRIP       # 4
NSUB = SSTRIP // 128        # 4
DCH = D // 128              # 32 contraction chunks
QTILE = 512                 # attention q-tile
NQT = S // QTILE            # 4
RD = QTILE // 128           # 4 key chunks per q-tile on the diagonal
NPCH = S // 128             # 16 key chunks

EXP_BIAS = -10.0            # exp(s-10): keeps exp in fp16 range; cancels
                            # in the softmax normalization


def _rope_tables():
    inv = 1.0 / (THETA ** (np.arange(0, HD, 2, dtype=np.float64) / HD))
    pos = np.arange(S, dtype=np.float64)
    freqs = pos[:, None] * inv[None, :]          # [S, 64]
    emb = np.concatenate([freqs, freqs], axis=1)  # [S, HD]
    cosT = np.cos(emb).T.astype(np.float16).copy()  # [HD, S]
    sinT = np.sin(emb).T.astype(np.float16).copy()
    return cosT, sinT


def _tri_mask():
    # tri[p, j] = 1 iff j >= p: the only masking a diagonal 128x128 score
    # block needs once QK/PV are column-restricted to the causal range.
    j = np.arange(128)[None, :]
    p = np.arange(128)[:, None]
    return (j >= p).astype(np.float16)


def build():
    nc = bacc.Bacc("TRN2", target_bir_lowering=False, debug=False,
                   enable_asserts=False, num_devices=NCORES)
    xt_d = nc.dram_tensor("xt", [D, S], F16, kind="ExternalInput").ap()
    wq_d = nc.dram_tensor("wq", [D, NQ], F16, kind="ExternalInput").ap()
    wk_d = nc.dram_tensor("wk", [D, NKV], F16, kind="ExternalInput").ap()
    wv_d = nc.dram_tensor("wv", [D, NKV], F16, kind="ExternalInput").ap()
    wo_d = nc.dram_tensor("wo", [NQ, D], F16, kind="ExternalInput").ap()
    out_d = nc.dram_tensor("out", [S, D], F16, kind="ExternalOutput").ap()

    cosT, sinT = _rope_tables()
    ident_d = nc.inline_tensor(
        np.eye(128, dtype=np.float16), "ident").ap()
    cos_d = nc.inline_tensor(cosT, "cosT").ap()
    sin_d = nc.inline_tensor(sinT, "sinT").ap()
    mask_d = nc.inline_tensor(_tri_mask(), "trimask").ap()

    with tile.TileContext(nc) as tc:
        _body(nc, tc, xt_d, wq_d, wk_d, wv_d, wo_d, out_d,
              ident_d, cos_d, sin_d, mask_d)
    nc.compile()
    return nc


def _phase1(nc, tc, xt_d, wq_d, wk_d, wv_d, ident_d, cos_d, sin_d,
            const_pool, qT_sb, kT_sb, vnat_sb):
    """QKV projection + RoPE. Strip 0 runs dc-outer (all 6 accumulators per
    x-chunk) so PE streams behind the DMA ramp instead of waiting for the
    whole strip; strips 1-3 run sweep-outer [q0..q3,k,v] so each
    accumulator's drain overlaps the following sweeps."""
    wqr = wq_d.rearrange("(c p) n -> p c n", p=128)
    wkr = wk_d.rearrange("(c p) n -> p c n", p=128)
    wvr = wv_d.rearrange("(c p) n -> p c n", p=128)
    xtr = xt_d.rearrange("(c p) s -> p c s", p=128)  # [128, DCH, S]

    with tc.tile_pool(name="rope_c", bufs=1) as rope_c, \
         tc.tile_pool(name="w1", bufs=1) as w1, \
         tc.tile_pool(name="xt0", bufs=3) as xt0_pool, \
         tc.tile_pool(name="xt", bufs=10) as xt_pool, \
         tc.tile_pool(name="p1tmp", bufs=2) as p1tmp, \
         tc.tile_pool(name="tp_ps", bufs=2, space="PSUM") as tp_ps, \
         tc.tile_pool(name="acc_ps", bufs=1, space="PSUM") as acc_ps:

        wq_sb = w1.tile([128, DCH, NQ], F16)
        wk_sb = w1.tile([128, DCH, NKV], F16)
        wv_sb = w1.tile([128, DCH, NKV], F16)

        # ---- strip 0 DMA ----
        # Each DMA_DIRECT2D costs ~1.2us of *issue* time on its engine
        # queue, so the ramp is gated by issue serialization, not
        # bandwidth. Spread issues across otherwise-idle queues (x/wk/wv
        # on sync, wq on scalar, constants on scalar) and keep the FIRST
        # transfers small so the first matmul's inputs land early.
        xsizes = [1, 3] + [4] * 7

        wgroups = [(0, 2), (2, 8), (8, 16), (16, 24), (24, 32)]
        xt0 = {}   # chunk -> (tile, offset)
        dc0 = 0
        for j, xg in enumerate(xsizes):
            t = xt0_pool.tile([128, xg, SSTRIP], F16, tag=f"x0_{min(j, 3)}",
                              bufs=1 if j < 3 else 3, name=f"x0_{j}")
            nc.sync.dma_start(t[:], xtr[:, dc0:dc0 + xg, 0:SSTRIP])
            for jj in range(xg):
                xt0[dc0 + jj] = (t, jj)
            dc0 += xg
            if j < len(wgroups):
                wsl = slice(*wgroups[j])
                nc.sync.dma_start(wk_sb[:, wsl, :], wkr[:, wsl, :])
                nc.scalar.dma_start(wq_sb[:, wsl, :], wqr[:, wsl, :])
                nc.scalar.dma_start(wv_sb[:, wsl, :], wvr[:, wsl, :])

        ident = const_pool.tile([128, 128], F16)
        nc.scalar.dma_start(ident[:], ident_d[:])
        cos_sb = rope_c.tile([128, S], F16)
        nc.scalar.dma_start(cos_sb[:], cos_d[:])
        sin_sb = rope_c.tile([128, S], F16)
        nc.scalar.dma_start(sin_sb[:], sin_d[:])

        XG = 4  # d-chunks per xt DMA for strips 1-3

        def load_xt(si, j):
            t = xt_pool.tile([128, XG, SSTRIP], F16, tag="xt",
                             name=f"xt{si}_{j}")
            nc.sync.dma_start(
                t[:], xtr[:, j * XG:(j + 1) * XG,
                          si * SSTRIP:(si + 1) * SSTRIP])
            return t

        def rope_store(src_ps, dst_ap, sslice):
            # dst = src*cos + rot(src)*sin, rot = [-src[64:], src[:64]].
            # SBUF+SBUF DVE operands must share their base partition, so
            # materialize the half-rotated src from PSUM first, then all
            # remaining ops are partition-aligned fp16 SBUF math.
            qrot = p1tmp.tile([128, SSTRIP], F16, tag="rope_qr",
                              name="rope_qr")
            nc.vector.tensor_copy(qrot[0:64, :], src_ps[64:128, :])
            nc.vector.tensor_copy(qrot[64:128, :], src_ps[0:64, :])
            qcos = p1tmp.tile([128, SSTRIP], F16, tag="rope_qc",
                              name="rope_qc")
            nc.vector.tensor_mul(qcos[:], src_ps[:], cos_sb[:, sslice])
            nc.vector.tensor_mul(qrot[:], qrot[:], sin_sb[:, sslice])
            nc.vector.tensor_sub(dst_ap[0:64, :], qcos[0:64, :],
                                 qrot[0:64, :])
            nc.vector.tensor_add(dst_ap[64:128, :], qcos[64:128, :],
                                 qrot[64:128, :])

        def vdrain(vacc, si):
            vstg = p1tmp.tile([128, SSTRIP], F16, tag="vstg")
            nc.vector.tensor_copy(vstg[:], vacc[:])
            for ss in range(NSUB):
                tp = tp_ps.tile([128, 128], F16, tag="tp")
                nc.tensor.transpose(tp[:], vstg[:, ss * 128:(ss + 1) * 128],
                                    ident[:])
                nc.vector.tensor_copy(vnat_sb[:, si * NSUB + ss, :], tp[:])

        # ---------------- strip 0: dc-outer ----------------
        qacc = [acc_ps.tile([128, SSTRIP], F32, tag=f"qacc{g}",
                            name=f"qacc{g}")
                for g in range(HQ)]
        kacc = acc_ps.tile([128, SSTRIP], F32, tag="kacc")
        vacc = acc_ps.tile([128, SSTRIP], F32, tag="vacc")

        def mm0(acc, wsl, dc):
            t, jj = xt0[dc]
            nc.tensor.matmul(acc[:], wsl, t[:, jj, :],
                             start=(dc == 0), stop=(dc == DCH - 1))

        # q0's last 4 chunks are hoisted ahead of the other accumulators'
        # so its RoPE drain overlaps their matmuls (strip 1 starts with the
        # q0 sweep and needs qacc0 drained).
        for dc in range(DCH - 4):
            mm0(kacc, wk_sb[:, dc, :], dc)
            mm0(vacc, wv_sb[:, dc, :], dc)
            for g in range(HQ):
                mm0(qacc[g], wq_sb[:, dc, g * 128:(g + 1) * 128], dc)
        for dc in range(DCH - 4, DCH):  # q0 tail first
            mm0(qacc[0], wq_sb[:, dc, 0:128], dc)
        rope_store(qacc[0], qT_sb[:, 0, 0:SSTRIP], slice(0, SSTRIP))
        for dc in range(DCH - 4, DCH):
            mm0(kacc, wk_sb[:, dc, :], dc)
            mm0(vacc, wv_sb[:, dc, :], dc)
            for g in range(1, HQ):
                mm0(qacc[g], wq_sb[:, dc, g * 128:(g + 1) * 128], dc)
        # vdrain first on the DVE queue: the PE transposes sit between
        # strip-0 and strip-1 matmuls, so vstg must not queue behind the
        # remaining RoPE drains.
        vdrain(vacc, 0)
        for g in range(1, HQ):
            rope_store(qacc[g], qT_sb[:, g, 0:SSTRIP], slice(0, SSTRIP))
        rope_store(kacc, kT_sb[:, 0:SSTRIP], slice(0, SSTRIP))

        # prefetch strip-1 x now, then wo is loaded by the caller
        xts = {}
        for j in range(DCH // XG):
            xts[(1, j)] = load_xt(1, j)

        # ---------------- strips 1-3: sweep-outer ----------------
        for si in range(1, NSTRIPS):
            s0 = si * SSTRIP
            sslice = slice(s0, s0 + SSTRIP)
            if si > 1:
                for j in range(DCH // XG):
                    xts[(si, j)] = load_xt(si, j)
            xtiles = [xts.pop((si, j)) for j in range(DCH // XG)]

            qacc = [acc_ps.tile([128, SSTRIP], F32, tag=f"qacc{g}",
                                name=f"qacc{g}")
                    for g in range(HQ)]
            kacc = acc_ps.tile([128, SSTRIP], F32, tag="kacc")
            vacc = acc_ps.tile([128, SSTRIP], F32, tag="vacc")

            def sweep(acc, wsl):
                for j in range(DCH // XG):
                    for jj in range(XG):
                        dc = j * XG + jj
                        nc.tensor.matmul(acc[:], wsl(dc),
                                         xtiles[j][:, jj, :],
                                         start=(dc == 0),
                                         stop=(dc == DCH - 1))

            def qsweep(g):
                sweep(qacc[g],
                      lambda dc: wq_sb[:, dc, g * 128:(g + 1) * 128])
                rope_store(qacc[g], qT_sb[:, g, sslice], sslice)

            def ksweep():
                sweep(kacc, lambda dc: wk_sb[:, dc, :])
                rope_store(kacc, kT_sb[:, sslice], sslice)

            def vsweep():
                sweep(vacc, lambda dc: wv_sb[:, dc, :])
                vdrain(vacc, si)

            if si < NSTRIPS - 1:
                for g in range(HQ):
                    qsweep(g)
                ksweep()
                vsweep()
            else:
                # last strip: k/v (and the v transposes) run mid-strip so
                # their PSUM banks and drains are long free when phase 2
                # reuses the banks; only the q3 rope trails the last matmul.
                qsweep(0)
                qsweep(1)
                qsweep(2)
                ksweep()
                vsweep()
                qsweep(3)


def _body(nc, tc, xt_d, wq_d, wk_d, wv_d, wo_d, out_d,
          ident_d, cos_d, sin_d, mask_d):
    with tc.tile_pool(name="const", bufs=1) as const_pool, \
         tc.tile_pool(name="persist", bufs=1) as persist:

        # persistent activations
        qT_sb = persist.tile([128, HQ, S], F16)    # [hd, head, s]
        kT_sb = persist.tile([128, S], F16)        # [hd, s]
        vnat_sb = persist.tile([128, NPCH, HD], F16)  # [s%128, s//128, hd]

        wo_pool_cm = tc.tile_pool(name="wo2", bufs=1)
        outh_pool_cm = tc.tile_pool(name="outh", bufs=1)
        wo_pool = wo_pool_cm.__enter__()
        outh_pool = outh_pool_cm.__enter__()
        wo_sb = wo_pool.tile([128, HQ, D], F16)
        outhT_sb = outh_pool.tile([128, HQ, S], F16)  # [hd, head, s]

        _phase1(nc, tc, xt_d, wq_d, wk_d, wv_d, ident_d, cos_d, sin_d,
                const_pool, qT_sb, kT_sb, vnat_sb)
        # wo prefetch: issued after phase-1 emission; the DMA engines run
        # well ahead of PE so this lands long before the first o_proj unit.
        nc.sync.dma_start(wo_sb[:],
                          wo_d.rearrange("(c p) m -> p c m", p=128))

        tri_sb = const_pool.tile([128, 128], F16)
        nc.sync.dma_start(tri_sb[:], mask_d[:])
        ebias = const_pool.tile([128, 1], F32)
        nc.gpsimd.memset(ebias[:], EXP_BIAS)
        ones_f = const_pool.tile([128, 1], F32)
        nc.gpsimd.memset(ones_f[:], 1.0)
        ones_col = const_pool.tile([128, 1], F16)
        nc.vector.tensor_copy(ones_col[:], ones_f[:])

        # -------- phase 2: attention with o_proj(qi-1) interleaved --------
        with tc.tile_pool(name="pt", bufs=2) as pt_pool, \
             tc.tile_pool(name="a2tmp", bufs=2) as a2tmp, \
             tc.tile_pool(name="osb", bufs=2) as osb_pool, \
             tc.tile_pool(name="st_ps", bufs=2, space="PSUM") as st_ps, \
             tc.tile_pool(name="oacc_ps", bufs=2, space="PSUM") as oacc_ps, \
             tc.tile_pool(name="opj_ps", bufs=2, space="PSUM") as opj_ps:

            osb_cur = {}

            def unit(si, mi):
                # one o_proj PSUM block: out rows [si*128,(si+1)*128),
                # cols [mi*512,(mi+1)*512); osb is split in column halves
                # so the DMA of each half starts earlier.
                hi = mi // 4
                if mi % 4 == 0:
                    osb_cur[si, hi] = osb_pool.tile([128, D // 2], F16,
                                                    tag="osb", name="osb")
                osb = osb_cur[si, hi]
                op = opj_ps.tile([128, 512], F32, tag="opj")
                for h in range(HQ):
                    nc.tensor.matmul(
                        op[:],
                        outhT_sb[:, h, si * 128:(si + 1) * 128],
                        wo_sb[:, h, mi * 512:(mi + 1) * 512],
                        start=(h == 0), stop=(h == HQ - 1))
                c0 = (mi % 4) * 512
                # alternate drain engines: the DVE is the loaded engine in
                # phase 2 and extra DVE work measurably throttles the chip
                if mi % 2 == 0:
                    nc.vector.tensor_copy(osb[:, c0:c0 + 512], op[:])
                else:
                    nc.scalar.copy(osb[:, c0:c0 + 512], op[:])
                if si == S // 128 - 1:
                    # last row-block: quarter-granularity stores so the
                    # final DMA (the kernel's tail) is as small as possible
                    if mi % 2 == 1:
                        q0_ = (mi % 4 - 1) * 512
                        nc.sync.dma_start(
                            out_d[si * 128:(si + 1) * 128,
                                  hi * (D // 2) + q0_:
                                  hi * (D // 2) + q0_ + 1024],
                            osb[:, q0_:q0_ + 1024])
                        if mi % 4 == 3:
                            del osb_cur[si, hi]
                elif mi % 4 == 3:
                    nc.sync.dma_start(
                        out_d[si * 128:(si + 1) * 128,
                              hi * (D // 2):(hi + 1) * (D // 2)],
                        osb[:])
                    del osb_cur[si, hi]

            pending_norms = []  # [(deadline_iter, fn)]

            def flush_norms(it=None):
                while pending_norms and (it is None
                                         or pending_norms[0][0] <= it):
                    pending_norms.pop(0)[1]()

            for qi in range(NQT):
                q0 = qi * QTILE
                npi = RD * (qi + 1)  # causal: key chunks [0, npi)
                units = [(si, mi) for si in range(qi * RD - RD, qi * RD)
                         for mi in range(D // 512)] if qi > 0 else []
                iters = [(h, pp) for h in range(HQ) for pp in range(npi // 2)]
                nit = len(iters)
                ucount = 0
                prev_pv = [None]

                for it, (h, pp) in enumerate(iters):
                    pi0 = 2 * pp
                    first_of_head = (pp == 0)
                    last_of_head = (pp == npi // 2 - 1)
                    oacc = oacc_ps.tile([128, QTILE], F32, tag="oacc",
                                        name=f"oacc{qi}_{h}") \
                        if first_of_head else prev_oacc
                    prev_oacc = oacc
                    pacc2 = a2tmp.tile([128, 2 * QTILE], F16, tag="pacc2",
                                       name=f"pacc2_{qi}_{h}") \
                        if first_of_head else prev_pacc2
                    prev_pacc2 = pacc2

                    # two key chunks share one [128,1024] PSUM tile and one
                    # exp ACTIVATE (amortizes ACT overhead). Diagonal
                    # chunks (r = pi - RD*qi >= 0) are column-restricted to
                    # the causally-valid range [r*128, 512): QK/PV skip the
                    # dead columns, exp skips the leading ones, and only
                    # the 128-wide boundary block needs a triangle mask.
                    # Skipped pt2 columns hold stale garbage that nothing
                    # downstream reads (exp'd stale scores stay finite in
                    # fp16 since scores are O(+-20)).
                    dpair = pp - 2 * qi  # last 2 pairs cross the diagonal
                    diag = dpair >= 0
                    rr = [(pi0 + k - RD * qi) * 128 if diag else 0
                          for k in range(2)]
                    st2 = st_ps.tile([128, 2 * QTILE], F32, tag="st2")
                    for k in range(2):
                        c = k * QTILE + rr[k]
                        nc.tensor.matmul(
                            st2[:, c:(k + 1) * QTILE],
                            kT_sb[:, (pi0 + k) * 128:(pi0 + k + 1) * 128],
                            qT_sb[:, h, q0 + rr[k]:q0 + QTILE],
                            start=True, stop=True)
                    pt2 = pt_pool.tile([128, 2 * QTILE], F16, tag="pt2")
                    if diag:
                        # two ACTs covering exactly the QK-written column
                        # ranges -- a single span would read PSUM bytes no
                        # QK wrote, which the dependency tracker flags as
                        # a conflict (and wedges the device)
                        nc.scalar.activation(pt2[:, rr[0]:QTILE],
                                             st2[:, rr[0]:QTILE],
                                             AF.Exp, bias=ebias[:])
                        nc.scalar.activation(pt2[:, QTILE + rr[1]:],
                                             st2[:, QTILE + rr[1]:],
                                             AF.Exp, bias=ebias[:])
                    else:
                        nc.scalar.activation(pt2[:], st2[:], AF.Exp,
                                             bias=ebias[:])
                    if diag:
                        for k in range(2):
                            c = k * QTILE + rr[k]
                            nc.vector.tensor_mul(pt2[:, c:c + 128],
                                                 pt2[:, c:c + 128],
                                                 tri_sb[:])
                    # denominator partial sums on DVE (replaces the
                    # per-pair ones-matmul pass on PE); restricted to the
                    # valid columns so stale pt2 never enters the sums
                    if first_of_head:
                        if diag:  # only qi == 0: zero the dead gap
                            nc.vector.tensor_copy(pacc2[:, 0:QTILE],
                                                  pt2[:, 0:QTILE])
                            nc.vector.memset(
                                pacc2[:, QTILE:QTILE + rr[1]], 0.0)
                            nc.vector.tensor_copy(
                                pacc2[:, QTILE + rr[1]:],
                                pt2[:, QTILE + rr[1]:])
                        else:
                            nc.vector.tensor_copy(pacc2[:], pt2[:])
                    elif diag:
                        for k in range(2):
                            c = k * QTILE + rr[k]
                            nc.vector.tensor_add(
                                pacc2[:, c:(k + 1) * QTILE],
                                pacc2[:, c:(k + 1) * QTILE],
                                pt2[:, c:(k + 1) * QTILE])
                    else:
                        nc.vector.tensor_add(pacc2[:], pacc2[:], pt2[:])

                    if prev_pv[0] is not None:
                        prev_pv[0]()
                        prev_pv[0] = None
                    # pending norms (previous heads) flush only after their
                    # oacc got its last PV accumulation (emission order is
                    # dependency order for Tile), and 3 iterations after
                    # their all_reduce so the ~3us gpsimd latency is covered
                    flush_norms(it)
                    if last_of_head:
                        # head's denominator: one ones-matmul pass over the
                        # accumulated probabilities (2 matmuls, vs a
                        # per-pair ones-matmul pass), into row 0 of a
                        # borrowed opj-ring PSUM slot. The broadcast runs
                        # on gpsimd one iteration later and the
                        # reciprocal+normalize three iterations later, so
                        # neither engine's latency blocks the DVE queue
                        # that feeds PV.
                        sacc = opj_ps.tile([128, 512], F32, tag="opj",
                                           name=f"sacc{qi}_{h}")
                        for k in range(2):
                            nc.tensor.matmul(
                                sacc[0:1, :], ones_col[:],
                                pacc2[:, k * QTILE:(k + 1) * QTILE],
                                start=(k == 0), stop=(k == 1))

                        def bcast(sacc=sacc, oacc=oacc, qi=qi, h=h, q0=q0,
                                  it=it):
                            srow = a2tmp.tile([1, QTILE], F32, tag="srow",
                                              name=f"srow{qi}_{h}")
                            nc.vector.tensor_copy(srow[:], sacc[0:1, :])
                            rb = a2tmp.tile([128, QTILE], F32, tag="rb",
                                            name=f"rb{qi}_{h}")
                            nc.gpsimd.partition_broadcast(rb[:], srow[:],
                                                          channels=128)
                            def norm(rb=rb, oacc=oacc, h=h, q0=q0):
                                nc.vector.reciprocal_approx_fast(rb[:],
                                                                 rb[:])
                                nc.vector.tensor_mul(
                                    outhT_sb[:, h, q0:q0 + QTILE],
                                    oacc[:], rb[:])
                            pending_norms.append((it + 3, norm))
                        pending_norms.append((it + 1, bcast))
                    while ucount * nit < (it + 1) * len(units):
                        unit(*units[ucount])
                        ucount += 1

                    def make_pv(oacc=oacc, pt2=pt2, pi0=pi0, npi=npi,
                                rr=rr):
                        def pv():
                            for k in range(2):
                                pi = pi0 + k
                                nc.tensor.matmul(
                                    oacc[:, rr[k]:],
                                    vnat_sb[:, pi, :],
                                    pt2[:, k * QTILE + rr[k]:
                                        (k + 1) * QTILE],
                                    start=(pi == 0), stop=(pi == npi - 1))
                        return pv
                    prev_pv[0] = make_pv()

                prev_pv[0]()
                prev_pv[0] = None
                flush_norms()
                while ucount < len(units):
                    unit(*units[ucount])
                    ucount += 1

            # o_proj for the last q-tile
            for si in range(S // 128 - RD, S // 128):
                for mi in range(D // 512):
                    unit(si, mi)

        outh_pool_cm.__exit__(None, None, None)
        wo_pool_cm.__exit__(None, None, None)


_NC_CACHE = None
LAST_RESULT = None
RUN_KWARGS = {}


def _get_nc():
    global _NC_CACHE
    if _NC_CACHE is None:
        _NC_CACHE = build()
    return _NC_CACHE


def kernel(x, wq, wk, wv, wo):
    global LAST_RESULT
    x = np.asarray(x, dtype=np.float32).reshape(S, D)
    xt = np.ascontiguousarray(x.T.astype(np.float16))
    wq = (np.asarray(wq, dtype=np.float32)
          * np.float32(1.0 / np.sqrt(HD))).astype(np.float16)
    wk = np.asarray(wk, dtype=np.float32).astype(np.float16)
    wv = np.asarray(wv, dtype=np.float32).astype(np.float16)
    wo = np.asarray(wo, dtype=np.float32).astype(np.float16)

    in_maps = []
    for c in range(NCORES):
        in_maps.append({
            "xt": xt,
            "wq": np.ascontiguousarray(wq[:, c * NQ:(c + 1) * NQ]),
            "wk": np.ascontiguousarray(wk[:, c * NKV:(c + 1) * NKV]),
            "wv": np.ascontiguousarray(wv[:, c * NKV:(c + 1) * NKV]),
            "wo": np.ascontiguousarray(wo[c * NQ:(c + 1) * NQ, :]),
        })

    nc = _get_nc()
    res = bass_utils.run_bass_kernel_spmd(nc, in_maps,
                                          core_ids=list(range(NCORES)),
                                          **RUN_KWARGS)
    LAST_RESULT = res
    acc = np.zeros((S, D), dtype=np.float64)
    for c in range(NCORES):
        acc += res.results[c]["out"].astype(np.float64)
    return acc.astype(np.float32).reshape(1, S, D)
